# revision 22
# baseline (speedup 1.0000x reference)
"""Trainium2 Bass kernel for a 3-layer GAT block (DeepGATBlockV2).

Strategy (8-core SPMD, nodes partitioned by dst range), v2 (bf16):
  - Per layer, each core builds bf16 table rows [x | a_s | a_d | pad] (256
    bf16 = 512B, the dma_gather granularity) for its 2500-node shard,
    AllGather -> full [N,256] bf16 table in DRAM.
  - Edges (incl. self loops) are sorted by dst on the host, bucketed into
    per-core dst-blocks of BLK nodes, padded to C chunks of 128 edge slots
    (pad slots: src=0, dstloc=-1 -> zero one-hot column).
  - Per block: dma_gather src rows (512B) from the gathered table and dst
    score-halves (256B) from the LOCAL shard table (dst rows are always
    local, so the dst gather does not wait on the AllGather).
  - Scores: q = a_s[src]+a_d[dst], w = exp(leaky_relu(q)) (no max-shift:
    |q| < 1 here); leaky via one fused scalar_tensor_tensor.
  - Per 128-edge chunk: S_h[e,n] = (iota==dloc)*w_h built by ONE fused DVE
    tensor_scalar per head (bf16), then PE (bf16, 1 cyc/row) accumulates
    ps_all[f, h*BLK+n] += Xg^T @ S and ps_den[0, h*BLK+n] += ones^T @ S.
  - Att post per block: y = ps_all * bcast(0.25/den), attT = sum_h W_h^T@y_h
    + bias + residual -> z buffer.
  - RMSNorm + FFN + RMSNorm batched over the whole 2500-col shard in 500-col
    strips (Act Rsqrt; 2 act-table loads per layer total).
"""

import numpy as np

import concourse.bass as bass
import concourse.bacc as bacc
import concourse.tile as tile
from concourse import mybir
from concourse.bass_utils import run_bass_kernel_spmd

F32 = mybir.dt.float32
BF16 = mybir.dt.bfloat16
I16 = mybir.dt.int16
AOT = mybir.AluOpType
ACT = mybir.ActivationFunctionType

EPS = 1.1920929e-07
NEG_SLOPE = 0.2
ROW = 256          # table row width in bf16 elems (512B)
SC0 = 128          # score offset within row
ABLATE = set()     # timing-ablation flags


def host_prep(inputs, cfg):
    """Returns (in_maps, C) -- per-core input dicts for the SPMD run."""
    N, E, CORES = cfg["N"], cfg["E"], cfg["CORES"]
    SHARD, BLK, BLOCKS = cfg["SHARD"], cfg["BLK"], cfg["BLOCKS"]
    L, D, H = cfg["L"], cfg["D"], cfg["H"]
    bf16 = mybir.dt.np(BF16)

    x = np.ascontiguousarray(np.asarray(inputs["x"], np.float32))
    ei = np.asarray(inputs["edge_index"], np.int64)
    src = ei[0]
    dst = ei[1]
    loops = np.arange(N, dtype=np.int64)
    src = np.concatenate([src, loops])
    dst = np.concatenate([dst, loops])
    order = np.argsort(dst, kind="stable")
    src, dst = src[order], dst[order]

    nblk_total = N // BLK
    blk_of = dst // BLK
    counts = np.bincount(blk_of, minlength=nblk_total)
    C = int(np.ceil(counts.max() / 128))
    cfg["C"] = C
    slots = C * 128

    # per (core, block) padded edge arrays in slot order (slot j of a block
    # -> gather output [j % 128, j // 128])
    srcs = np.zeros((CORES, BLOCKS, slots), np.int64)
    dstl = np.zeros((CORES, BLOCKS, slots), np.int64)  # SHARD-local dst rows
    dloc = np.full((CORES, BLOCKS, slots), -1.0, np.float32)
    starts = np.concatenate([[0], np.cumsum(counts)])
    for b in range(nblk_total):
        core, blk = b // BLOCKS, b % BLOCKS
        s, e = int(starts[b]), int(starts[b + 1])
        n = e - s
        srcs[core, blk, :n] = src[s:e]
        dstl[core, blk, :n] = dst[s:e] - core * SHARD
        dstl[core, blk, n:] = (b % BLOCKS) * BLK  # valid local row for pads
        dloc[core, blk, :n] = (dst[s:e] - b * BLK).astype(np.float32)

    def wrap_idx(a):
        # a: [BLOCKS, slots] int -> int16 [128, BLOCKS * slots//16], slot j of
        # block b at [j % 16 (replicated x8), b*slots//16 + j//16]
        a16 = a.reshape(cfg["BLOCKS"], slots // 16, 16).transpose(0, 2, 1)
        a16 = a16.reshape(1, cfg["BLOCKS"] * 16, slots // 16)
        cols = np.concatenate(
            [a16[0, b * 16:(b + 1) * 16, :] for b in range(cfg["BLOCKS"])],
            axis=1)  # [16, BLOCKS*slots//16]
        assert a.max() < 2 ** 15
        return np.tile(cols.astype(np.int16), (8, 1))

    # dloc layout [128, BLOCKS*C]: [p, b*C + ch] = slot ch*128+p of block b
    dloc_t = dloc.reshape(CORES, BLOCKS, C, 128).transpose(0, 3, 1, 2) \
                 .reshape(CORES, 128, BLOCKS * C)

    Wg = np.asarray(inputs["W_gat"], np.float32)     # [L, D, H*D]
    a_s = np.asarray(inputs["att_src"], np.float32)  # [L, H, D]
    a_d = np.asarray(inputs["att_dst"], np.float32)
    wasd = np.zeros((L, D, 2 * H), np.float32)
    for l in range(L):
        for h in range(H):
            Wh = Wg[l][:, h * D:(h + 1) * D]
            wasd[l, :, h] = Wh @ a_s[l, h]
            wasd[l, :, H + h] = Wh @ a_d[l, h]

    def col3(name):  # [L, D] -> [D, L] f32
        return np.ascontiguousarray(np.asarray(inputs[name], np.float32).T)

    def row3b(name):  # [L, D] -> [1, L*D] bf16
        return np.ascontiguousarray(
            np.asarray(inputs[name], np.float32).reshape(1, -1).astype(bf16))

    common = {
        "wasd": np.ascontiguousarray(wasd),
        "wgat": np.ascontiguousarray(Wg.astype(bf16)),
        "w1": np.ascontiguousarray(
            np.asarray(inputs["W1"], np.float32).astype(bf16)),
        "w2": np.ascontiguousarray(
            np.asarray(inputs["W2"], np.float32).astype(bf16)),
        "bg": col3("bias_gat"), "b1": col3("b1"), "b2": col3("b2"),
        "n1": row3b("norm1_w"), "n2": row3b("norm2_w"),
        "iota": np.ascontiguousarray(
            np.tile(np.arange(BLK, dtype=np.float32), (128, 1)).astype(bf16)),
        "ident": np.eye(128, dtype=np.float32),
        "ones": np.ones((128, 128), np.float32).astype(bf16),
        # hsel[k, h*128+m] = 0.25 * (k == h): bcast-matmul selector that
        # replicates row h of a [H, BLK] tile to 128 partitions, x0.25
        "hsel": np.ascontiguousarray(
            (0.25 * np.eye(H, dtype=np.float32)).repeat(128, axis=1)
            .astype(bf16)),
    }
    in_maps = []
    for c in range(CORES):
        m = dict(common)
        m["xin"] = np.ascontiguousarray(x[c * SHARD:(c + 1) * SHARD])
        m["idxa"] = wrap_idx(srcs[c])
        m["idxb"] = wrap_idx(dstl[c])
        m["dloc"] = np.ascontiguousarray(dloc_t[c])
        in_maps.append(m)
    return in_maps, C


def build_program(cfg, debug=False):
    N, CORES = cfg["N"], cfg["CORES"]
    SHARD, BLK, BLOCKS, C = cfg["SHARD"], cfg["BLK"], cfg["BLOCKS"], cfg["C"]
    L, D, H = cfg["L"], cfg["D"], cfg["H"]
    slots = C * 128
    HB = H * BLK
    NS = 500                        # norm strip width (cols)
    NSTRIPS = SHARD // NS
    RP = cfg.get("REPS", 1)

    nc = bacc.Bacc("TRN2", target_bir_lowering=False, debug=debug,
                   num_devices=CORES, num_swdge_queues=4,
                   dynamic_dma_scratch_size=65536)

    def din(name, shape, dt=F32):
        return nc.dram_tensor(name, list(shape), dt, kind="ExternalInput").ap()

    xin = din("xin", (SHARD, D))
    idxa = din("idxa", (128, BLOCKS * slots // 16), I16)
    idxb = din("idxb", (128, BLOCKS * slots // 16), I16)
    dloc = din("dloc", (128, BLOCKS * C))
    wasd = din("wasd", (L, D, 2 * H))
    wgat = din("wgat", (L, D, H * D), BF16)
    w1 = din("w1", (L, D, D), BF16)
    w2 = din("w2", (L, D, D), BF16)
    bg, b1, b2 = din("bg", (D, L)), din("b1", (D, L)), din("b2", (D, L))
    n1, n2 = din("n1", (1, L * D), BF16), din("n2", (1, L * D), BF16)
    iota_i = din("iota", (128, BLK), BF16)
    ident_i = din("ident", (128, 128))
    ones_i = din("ones", (128, 128), BF16)
    hsel_i = din("hsel", (H, H * 128), BF16)
    out = nc.dram_tensor("out", [SHARD, D], F32, kind="ExternalOutput").ap()

    with tile.TileContext(nc) as tc:
        with tc.tile_pool(name="persist", bufs=1) as pp, \
             tc.tile_pool(name="dram", bufs=1, space="DRAM") as dp, \
             tc.tile_pool(name="gath", bufs=3) as gp, \
             tc.tile_pool(name="sc", bufs=3) as scp, \
             tc.tile_pool(name="chunk", bufs=6) as cp, \
             tc.tile_pool(name="post", bufs=2) as pop, \
             tc.tile_pool(name="psA", bufs=1, space="PSUM") as psA, \
             tc.tile_pool(name="psB", bufs=1, space="PSUM") as psB:

            # ---- persistent SBUF ----
            idxa_s = pp.tile([128, BLOCKS * slots // 16], I16)
            idxb_s = pp.tile([128, BLOCKS * slots // 16], I16)
            dloc_s = pp.tile([128, BLOCKS * C], F32)
            wasd_s = pp.tile([128, L * 2 * H], F32)
            wgat_s = pp.tile([128, L * H * D], BF16)
            w1_s = pp.tile([128, L * D], BF16)
            w2_s = pp.tile([128, L * D], BF16)
            bg_s = pp.tile([128, L], F32)
            b1_s = pp.tile([128, L], F32)
            b2_s = pp.tile([128, L], F32)
            n1_s = pp.tile([1, L * D], BF16)
            n2_s = pp.tile([1, L * D], BF16)
            iota_s = pp.tile([128, BLK], BF16)
            ident_s = pp.tile([128, 128], F32)
            ones_s = pp.tile([128, 128], BF16)
            hsel_s = pp.tile([H, H * 128], BF16)
            scl_s = pp.tile([128, 1], BF16)     # 1/D for the mean matmul
            xT = pp.tile([128, SHARD], F32)
            zbuf = pp.tile([128, SHARD], F32)
            zn1b = pp.tile([128, SHARD], BF16)
            eps_s = pp.tile([1, 1], F32)

            dma = nc.sync.dma_start
            dma(idxa_s[:], idxa)
            dma(idxb_s[:], idxb)
            dma(dloc_s[:], dloc)
            for l in range(L):
                dma(wasd_s[:, l * 2 * H:(l + 1) * 2 * H], wasd[l])
                dma(wgat_s[:, l * H * D:(l + 1) * H * D], wgat[l])
                dma(w1_s[:, l * D:(l + 1) * D], w1[l])
                dma(w2_s[:, l * D:(l + 1) * D], w2[l])
            dma(bg_s[:], bg)
            dma(b1_s[:], b1)
            dma(b2_s[:], b2)
            dma(n1_s[:], n1)
            dma(n2_s[:], n2)
            dma(iota_s[:], iota_i)
            dma(ident_s[:], ident_i)
            dma(ones_s[:], ones_i)
            dma(hsel_s[:], hsel_i)
            nc.vector.memset(eps_s[:], EPS)
            nc.vector.memset(scl_s[:], 1.0 / D)

            # ---- DRAM tables for gather + collective ----
            tshard = dp.tile([SHARD, ROW], BF16)
            if CORES > 1:
                tfulls = [dp.tile([N, ROW], BF16, addr_space="Shared",
                                  tag=f"tfull{i}", name=f"tfull{i}")
                          for i in range(L * RP)]
            else:
                tfulls = [tshard] * (L * RP)

            # ---- init: transpose input shard to feature-major xT ----
            for b in range(BLOCKS):
                xr = gp.tile([BLK, D], F32, tag="xr")
                nc.sync.dma_start(xr[:], xin[b * BLK:(b + 1) * BLK, :])
                ps_t = psB.tile([D, BLK], F32, tag="pb", bufs=2)
                nc.tensor.transpose(ps_t[:], xr[:], ident_s[:BLK, :BLK])
                nc.scalar.copy(xT[:, b * BLK:(b + 1) * BLK], ps_t[:])

            for rep in range(RP):
             for l in range(L):
                # ---- phase A: build bf16 table rows [x | a_s | a_d | 0] ----
                for b in range(BLOCKS):
                    xb = xT[:, b * BLK:(b + 1) * BLK]
                    ps_a = psB.tile([BLK, 2 * H], F32, tag="pb", bufs=2)
                    nc.tensor.matmul(ps_a[:], xb,
                                     wasd_s[:, l * 2 * H:(l + 1) * 2 * H],
                                     start=True, stop=True)
                    ps_x = psB.tile([BLK, D], F32, tag="pb", bufs=2)
                    nc.tensor.transpose(ps_x[:], xb, ident_s[:])
                    tt = gp.tile([BLK, ROW], BF16, tag="tt")
                    nc.scalar.copy(tt[:, 0:D], ps_x[:])
                    nc.scalar.copy(tt[:, SC0:SC0 + 2 * H], ps_a[:])
                    nc.vector.memset(tt[:, SC0 + 2 * H:ROW], 0.0)
                    nc.sync.dma_start(tshard[b * BLK:(b + 1) * BLK, :], tt[:])

                # ---- phase B: AllGather ----
                if CORES > 1 and "coll" not in ABLATE:
                    nc.gpsimd.collective_compute(
                        "AllGather", AOT.bypass,
                        replica_groups=[list(range(CORES))],
                        ins=[tshard.opt()], outs=[tfulls[rep * L + l].opt()])
                tfull = tfulls[rep * L + l]

                # ---- phase C: per-block edge aggregation ----
                def chunk_phase(b):
                    ga = gp.tile([128, C * ROW], BF16, tag="ga")
                    gb = gp.tile([128, C * SC0], BF16, tag="gb")
                    ic0 = b * (slots // 16)
                    ic1 = (b + 1) * (slots // 16)
                    # ablation: shrink to one chunk's worth of indices so
                    # tiles are still written (tile framework requirement)
                    # split each gather across two SWDGE queues for ring
                    # parallelism; ablation shrinks to one chunk per half
                    hs = slots // 2            # slot split (multiple of 128)
                    hi = hs // 16              # idx-column split
                    hc = (hs // 128) * ROW     # ga column split
                    hcb = (hs // 128) * SC0    # gb column split
                    ni = 128 if "ga" in ABLATE else hs
                    nch = ni // 128
                    for half, qa, qb in ((0, 0, 1), (1, 2, 3)):
                        icm = ic0 + half * hi
                        nc.gpsimd.dma_gather(
                            ga[:, half * hc:half * hc + nch * ROW]
                            .rearrange("p (c e) -> p c e", e=ROW),
                            tfull[:], idxa_s[:, icm:icm + hi],
                            num_idxs=ni, num_idxs_reg=ni,
                            elem_size=ROW, queue_num=qa, single_packet=False)
                        nc.gpsimd.dma_gather(
                            gb[:, half * hcb:half * hcb + nch * SC0]
                            .rearrange("p (c e) -> p c e", e=SC0),
                            tshard[:, SC0:ROW], idxb_s[:, icm:icm + hi],
                            num_idxs=ni, num_idxs_reg=ni,
                            elem_size=SC0, elem_step=ROW, queue_num=qb,
                            single_packet=False)
                    gav = ga[:].rearrange("p (c e) -> p c e", e=ROW)
                    gbv = gb[:].rearrange("p (c e) -> p c e", e=SC0)

                    q = scp.tile([128, C * H], BF16, tag="q")
                    el = scp.tile([128, C * H], BF16, tag="el")
                    wex = scp.tile([128, C * H], BF16, tag="wex")
                    if "score" not in ABLATE:
                        nc.vector.tensor_add(
                            q[:].rearrange("p (c h) -> p c h", h=H),
                            gav[:, :, SC0:SC0 + H], gbv[:, :, H:2 * H])
                        nc.vector.scalar_tensor_tensor(
                            el[:], q[:], NEG_SLOPE, q[:],
                            op0=AOT.mult, op1=AOT.max)
                        nc.scalar.activation(wex[:], el[:], ACT.Exp)
                    else:
                        nc.vector.memset(wex[:], 1.0)

                    ps_all = psA.tile([D, HB], F32, tag="ps_all",
                                      name=f"ps_all_{rep}_{l}_{b}", bufs=2)
                    ps_den = psA.tile([H, BLK], F32, tag="ps_den",
                                      name=f"ps_den_{rep}_{l}_{b}", bufs=2)
                    nheads = 1 if "sdve" in ABLATE else H
                    nchunks = 1 if "smm" in ABLATE else C
                    for ch in range(C):
                        dcol = dloc_s[:, b * C + ch:b * C + ch + 1]
                        oh = cp.tile([128, BLK], BF16, tag="oh")
                        sh = cp.tile([128, HB], BF16, tag="sh")
                        nc.vector.tensor_scalar(oh[:], iota_s[:], dcol,
                                                None, AOT.is_equal)
                        # sh[p, h*BLK+n] = oh[p, n] * wex[p, ch*H+h]
                        ohb = oh[:].rearrange("p (c n) -> p c n", c=1) \
                                   .broadcast_to([128, H, BLK])
                        wxb = wex[:, ch * H:(ch + 1) * H] \
                            .rearrange("p (h c) -> p h c", c=1) \
                            .broadcast_to([128, H, BLK])
                        nc.vector.tensor_tensor(
                            sh[:].rearrange("p (h n) -> p h n", n=BLK)
                            [:, 0:nheads, :],
                            ohb[:, 0:nheads, :], wxb[:, 0:nheads, :],
                            op=AOT.mult)
                        if ch < nchunks:
                            nc.tensor.matmul(ps_all[:], gav[:, ch, 0:D],
                                             sh[:], start=(ch == 0),
                                             stop=(ch == nchunks - 1))
                            nc.tensor.matmul(ps_den[:],
                                             wex[:, ch * H:(ch + 1) * H],
                                             oh[:], start=(ch == 0),
                                             stop=(ch == nchunks - 1))
                    return ps_all, ps_den

                def att_post(b, ps_all, ps_den):
                    if "post" in ABLATE:
                        return
                    rden = pop.tile([H, BLK], BF16, tag="rden")
                    with nc.allow_low_precision(reason="bf16 alpha ok"):
                        nc.vector.reciprocal(rden[:], ps_den[:])
                    # rb[f, h*BLK+n] = 0.25 / den[h, n] (head-mean folded in)
                    ps_rb = psB.tile([128, HB], F32, tag="pb", bufs=2)
                    for h in range(H):
                        nc.tensor.matmul(
                            ps_rb[:, h * BLK:(h + 1) * BLK],
                            hsel_s[:, h * 128:(h + 1) * 128], rden[:],
                            start=(h == 0), stop=(h == H - 1))
                    rb = pop.tile([128, HB], BF16, tag="rb")
                    nc.scalar.copy(rb[:], ps_rb[:])
                    yh = pop.tile([128, HB], BF16, tag="yh")
                    nc.vector.tensor_mul(yh[:], ps_all[:], rb[:])
                    ps_att = psB.tile([D, BLK], F32, tag="pb", bufs=2)
                    for h in range(H):
                        nc.tensor.matmul(
                            ps_att[:],
                            wgat_s[:, (l * H + h) * D:(l * H + h + 1) * D],
                            yh[:, h * BLK:(h + 1) * BLK],
                            start=(h == 0), stop=(h == H - 1))
                    xb = xT[:, b * BLK:(b + 1) * BLK]
                    zb = zbuf[:, b * BLK:(b + 1) * BLK]
                    nc.vector.scalar_tensor_tensor(
                        zb, ps_att[:], bg_s[:, l:l + 1], xb,
                        op0=AOT.add, op1=AOT.add)

                pending = None
                for b in range(BLOCKS):
                    if pending is not None:
                        att_post(*pending)
                    handles = chunk_phase(b)
                    pending = (b, *handles)
                att_post(*pending)

                # ---- phase D: batched RMSNorm + FFN + RMSNorm ----
                def rms_strip(z_ap, nw_row, xout, cols, tag):
                    """xout[:, cols] = rmsnorm(z_ap) * nw, written as f32 or
                    bf16 depending on xout dtype."""
                    zsq = pop.tile([D, NS], BF16, tag=f"zsq{tag}")
                    nc.scalar.activation(zsq[:], z_ap, ACT.Square)
                    ps_ss = psB.tile([1, NS], F32, tag="pb_ss", bufs=2)
                    nc.tensor.matmul(ps_ss[:], scl_s[:], zsq[:],
                                     start=True, stop=True)
                    srt = pop.tile([1, NS], F32, tag=f"srt{tag}")
                    nc.scalar.activation(srt[:], ps_ss[:], ACT.Sqrt,
                                         bias=eps_s[:])
                    rin = pop.tile([1, NS], BF16, tag=f"rin{tag}")
                    with nc.allow_low_precision(reason="bf16 norm ok"):
                        nc.vector.reciprocal(rin[:], srt[:])
                    ps_rn = psB.tile([D, NS], F32, tag="pb", bufs=2)
                    nc.tensor.matmul(ps_rn[:], nw_row, rin[:],
                                     start=True, stop=True)
                    nc.vector.tensor_mul(xout, z_ap, ps_rn[:])

                if "post" not in ABLATE:
                    for s in range(NSTRIPS):
                        cols = slice(s * NS, (s + 1) * NS)
                        rms_strip(zbuf[:, cols],
                                  n1_s[0:1, l * D:(l + 1) * D],
                                  zn1b[:, cols], cols, "a")
                        ps_f1 = psB.tile([D, NS], F32, tag="pb", bufs=2)
                        nc.tensor.matmul(ps_f1[:], w1_s[:, l * D:(l + 1) * D],
                                         zn1b[:, cols], start=True, stop=True)
                        f1 = pop.tile([D, NS], BF16, tag="f1")
                        nc.scalar.activation(f1[:], ps_f1[:], ACT.Relu,
                                             bias=b1_s[:, l:l + 1])
                        ps_f2 = psB.tile([D, NS], F32, tag="pb", bufs=2)
                        nc.tensor.matmul(ps_f2[:], w2_s[:, l * D:(l + 1) * D],
                                         f1[:], start=True, stop=True)
                        # z3 = ps_f2 + b2 + zn1
                        z3 = zbuf[:, cols]
                        nc.vector.scalar_tensor_tensor(
                            z3, ps_f2[:], b2_s[:, l:l + 1], zn1b[:, cols],
                            op0=AOT.add, op1=AOT.add)
                        rms_strip(z3, n2_s[0:1, l * D:(l + 1) * D],
                                  xT[:, cols], cols, "b")

            # ---- output: transpose back to node-major ----
            for b in range(BLOCKS):
                ps_o = psB.tile([BLK, D], F32, tag="pb", bufs=2)
                nc.tensor.transpose(ps_o[:], xT[:, b * BLK:(b + 1) * BLK],
                                    ident_s[:])
                ot = gp.tile([BLK, D], F32, tag="ot")
                nc.scalar.copy(ot[:], ps_o[:])
                nc.sync.dma_start(out[b * BLK:(b + 1) * BLK, :], ot[:])

    nc.compile()
    return nc


FULL_CFG = dict(N=20000, E=320000, CORES=8, SHARD=2500, BLK=125, BLOCKS=20,
                C=None, L=3, D=128, H=4)


def kernel_run(inputs, trace=False):
    cfg = dict(FULL_CFG)
    in_maps, C = host_prep(inputs, cfg)
    nc = build_program(cfg)
    res = run_bass_kernel_spmd(nc, in_maps, list(range(cfg["CORES"])),
                               trace=trace)
    out = np.concatenate([r["out"] for r in res.results], axis=0)
    return out, res


def kernel(**inputs):
    out, _ = kernel_run(inputs)
    return out.astype(np.float32)
